# revision 1
# baseline (speedup 1.0000x reference)
"""DenseStructuralGAT v5: single merged kernel, j-major, no PE transposes.

Per core c (owns output rows i in [c*1024, (c+1)*1024)):
  inputs (host-staged):
    mt  [N, RB]  fp16  = M[c*RB:(c+1)*RB, :].T          (j-major)
    xt  [FIN, N] fp16  = X.T                             (replicated)
    xtc [FIN, RB] fp16 = X.T[:, c*RB:(c+1)*RB]           (own block)
    wt  [FIN, FOUT] fp16 = W.T ; wf [FOUT, FIN] fp16 = W (replicated)
    attn [1, 2*FOUT] f32 ; reps [1,1] int32 (runtime rep count)
  output: zc [RB, FOUT] f32

Pipeline per rep:
  setup (once): war/wal = W^T a_{r,l} via PE; combined rhs wtc=[WT|war|wal]
  s-pass: s_bcast[128, RB] = broadcast of s_i = X_own @ wal over partitions
  H-pass: for all 64 j-tiles: [H | t | s] = XT_tile^T @ wtc  (PE, fp16)
  chain per j-tile (128 j x 1024 i), engines parametrized:
    e = prelu(s_bcast + t_j)     [ACT]   or  STT pair on DVE
    v = mt * e                   [DVE]
    z = (mt==0)*-250             [GPSIMD or DVE]
    u = v + z                    [DVE]
    p = exp(u)                   [ACT, fp16]
  PE per j-tile: rowsum (ones matmul) + Z^T accumulation (2 psum banks)
  finale: Z = sigmoid(Z^T.T / rowsum)  -> dma out
"""

import numpy as np
import ml_dtypes

import concourse.bacc as bacc
import concourse.mybir as mybir
import concourse.tile as tile
from concourse.bass_utils import run_bass_kernel_spmd
from concourse.masks import make_identity

N, FIN, FOUT = 8192, 512, 256
NCORES = 8
RB = N // NCORES          # 1024 rows per core
NJT = N // 128            # 64 j-tiles
NIT = RB // 128           # 8 i-tiles
NKT = FIN // 128          # 4 k-tiles
NEG = -250.0
ALPHA = 0.2

F32 = mybir.dt.float32
F16 = mybir.dt.float16
I32 = mybir.dt.int32
AF = mybir.ActivationFunctionType
ALU = mybir.AluOpType


def default_sched():
    """Per-jt engine choices: (leaky_on_act, z_engine, hcopy_on_act)."""
    sched = []
    for jt in range(NJT):
        leaky_act = (jt % 8) < 5         # 5/8 prelu on ACT, rest DVE STT
        z_gp = False                     # GPSIMD z-ops measured far too slow
        hcopy_act = (jt % 8) < 3         # 3/8 H-copies on ACT
        sched.append((leaky_act, z_gp, hcopy_act))
    return sched


def build_v5(reps=2, sched=None, xt_dtype=F16):
    """Static rep count; reps=2 doubles as the correctness kernel."""
    if sched is None:
        sched = default_sched()
    nc = bacc.Bacc()
    mt_d = nc.dram_tensor("mt", [N, RB], F16, kind="ExternalInput")
    xt_d = nc.dram_tensor("xt", [FIN, N], xt_dtype, kind="ExternalInput")
    xtc_d = nc.dram_tensor("xtc", [FIN, RB], F16, kind="ExternalInput")
    wt_d = nc.dram_tensor("wt", [FIN, FOUT], F16, kind="ExternalInput")
    wf_d = nc.dram_tensor("wf", [FOUT, FIN], F16, kind="ExternalInput")
    a_d = nc.dram_tensor("attn", [1, 2 * FOUT], F32, kind="ExternalInput")
    z_d = nc.dram_tensor("zc", [RB, FOUT], F32, kind="ExternalOutput")

    with tile.TileContext(nc) as tc:
        with (
            tc.tile_pool(name="cst", bufs=1) as cst,
            tc.tile_pool(name="zps", bufs=1, space="PSUM") as zpool,
            tc.tile_pool(name="hps", bufs=2, space="PSUM") as hpool,
            tc.tile_pool(name="mtp", bufs=3) as mtp,
            tc.tile_pool(name="xtp", bufs=2) as xtp,
            tc.tile_pool(name="chn", bufs=3) as chp,
            tc.tile_pool(name="pch", bufs=8) as ppool,
            tc.tile_pool(name="msc", bufs=2) as misc,
            tc.tile_pool(name="fin", bufs=1) as fin,
            tc.tile_pool(name="fin2", bufs=2) as fin2,
        ):
            # ---------------- constants / setup (outside rep loop)
            ident32 = cst.tile([128, 128], F32, tag="id32")
            make_identity(nc, ident32[:])
            ident16 = cst.tile([128, 128], F16, tag="id16")
            make_identity(nc, ident16[:])
            one11 = cst.tile([1, 1], F32, tag="one11")
            nc.vector.memset(one11[:], 1.0)
            onescol = cst.tile([128, 1], F16, tag="onescol")
            nc.vector.memset(onescol[:], 1.0)
            ones128 = cst.tile([128, 128], F16, tag="ones128")
            nc.vector.memset(ones128[:], 1.0)

            a_sb = cst.tile([1, 2 * FOUT], F32, tag="asb")
            nc.sync.dma_start(a_sb[:], a_d[:])
            wf_sb = cst.tile([128, 2, FIN], F16, tag="wfsb")
            nc.sync.dma_start(
                wf_sb[:], wf_d[:].rearrange("(ft p) k -> p ft k", p=128)
            )
            wt_sb = cst.tile([128, NKT, FOUT], F16, tag="wtsb")
            nc.sync.dma_start(
                wt_sb[:], wt_d[:].rearrange("(kt p) f -> p kt f", p=128)
            )
            xtc_sb = cst.tile([128, NKT, RB], F16, tag="xtcsb")
            nc.scalar.dma_start(
                xtc_sb[:], xtc_d[:].rearrange("(kt p) i -> p kt i", p=128)
            )

            # attention halves as fp16 columns: [a_l0 a_l1 a_r0 a_r1]
            a_cols = cst.tile([128, 4], F16, tag="acols")
            for h in range(4):
                pa = hpool.tile([128, 4], F32, tag="hps2")
                nc.tensor.matmul(
                    pa[:, 0:1], a_sb[0:1, 128 * h:128 * (h + 1)], one11[:]
                )
                nc.any.tensor_copy(a_cols[:, h:h + 1], pa[:, 0:1])

            # wrs[kc] = (war | wal) per 128-k slice
            wrs = cst.tile([128, NKT, 2], F16, tag="wrs")
            for kc in range(NKT):
                pst = hpool.tile([128, 2], F32, tag="hps2")
                for ft in range(2):
                    rhs = misc.tile([128, 2], F16, tag="arhs")
                    nc.vector.tensor_copy(rhs[:, 0:1], a_cols[:, 2 + ft:3 + ft])
                    nc.vector.tensor_copy(rhs[:, 1:2], a_cols[:, ft:ft + 1])
                    nc.tensor.matmul(
                        pst[:],
                        wf_sb[:, ft, kc * 128:(kc + 1) * 128],
                        rhs[:],
                        start=(ft == 0), stop=(ft == 1),
                    )
                nc.any.tensor_copy(wrs[:, kc, :], pst[:])

            # combined H-pass rhs: [WT(256) | war | wal] per k-tile
            wtc = cst.tile([128, NKT, FOUT + 2], F16, tag="wtc")
            for kt in range(NKT):
                nc.vector.tensor_copy(wtc[:, kt, :FOUT], wt_sb[:, kt, :])
                nc.vector.tensor_copy(wtc[:, kt, FOUT:], wrs[:, kt, :])

            # persistent accumulators
            zps = [
                zpool.tile([128, RB], F32, tag=f"zps{fc}", name=f"zps{fc}")
                for fc in range(2)
            ]
            rsT = zpool.tile([1, RB], F32, tag="rsT", name="rsT")

            h_sb = cst.tile([128, NJT, FOUT], F16, tag="hsb")
            t_sb = cst.tile([128, NJT], F32, tag="tsb")
            s_bcast = cst.tile([128, RB], F16, tag="sbcast")

            # runtime rep counts need all-engine ScalarValues (For_i
            # asserts); static reps only.
            loop = tc.For_i(0, reps, 1)
            with loop:
                # ---- s-pass: s columns from own XT block
                s_cols = misc.tile([128, NIT], F32, tag="scols")
                for it in range(NIT):
                    ps_s = hpool.tile([128, 2], F32, tag="hps2")
                    for kt in range(NKT):
                        nc.tensor.matmul(
                            ps_s[:, 0:1],
                            xtc_sb[:, kt, it * 128:(it + 1) * 128],
                            wrs[:, kt, 1:2],
                            start=(kt == 0), stop=(kt == NKT - 1),
                        )
                    nc.any.tensor_copy(s_cols[:, it:it + 1], ps_s[:, 0:1])
                # broadcast over partitions: ones128^T @ diag(s_col) has
                # every partition row equal to s_col (free dim = i)
                for it in range(NIT):
                    diag = misc.tile([128, 128], F16, tag="sdiag")
                    nc.vector.tensor_scalar(
                        diag[:], ident16[:], s_cols[:, it:it + 1], None,
                        ALU.mult)
                    ps_b = hpool.tile([128, 258], F32, tag="hps2")
                    nc.tensor.matmul(ps_b[:, :128], ones128[:], diag[:])
                    nc.any.tensor_copy(
                        s_bcast[:, it * 128:(it + 1) * 128], ps_b[:, :128]
                    )

                # ---- H-pass + chain + PE accumulation, 8 j-tiles per chunk
                for ct in range(8):
                    xt_ch = xtp.tile([128, NKT, 1024], xt_dtype, tag="xtch")
                    nc.scalar.dma_start(
                        xt_ch[:],
                        xt_d[:].rearrange("(kt p) j -> p kt j", p=128)
                        [:, :, ct * 1024:(ct + 1) * 1024],
                    )
                    mt_ch = mtp.tile([128, 8, RB], F16, tag="mtch")
                    nc.sync.dma_start(
                        mt_ch[:],
                        mt_d[:].rearrange("(jt p) i -> p jt i", p=128)
                        [:, ct * 8:(ct + 1) * 8, :],
                    )
                    for js in range(8):
                        jt = ct * 8 + js
                        leaky_act, z_gp, hcopy_act = sched[jt]
                        ph = hpool.tile([128, 258], F32, tag="hps2")
                        for kt in range(NKT):
                            nc.tensor.matmul(
                                ph[:],
                                xt_ch[:, kt, js * 128:(js + 1) * 128],
                                wtc[:, kt, :],
                                start=(kt == 0), stop=(kt == NKT - 1),
                            )
                        if hcopy_act:
                            nc.scalar.activation(
                                h_sb[:, jt, :], ph[:, :FOUT], AF.Copy)
                        else:
                            nc.vector.tensor_copy(h_sb[:, jt, :], ph[:, :FOUT])
                        nc.any.tensor_copy(t_sb[:, jt:jt + 1], ph[:, FOUT:FOUT + 1])

                        mt_t = mt_ch[:, js, :]
                        if leaky_act:
                            e_t = chp.tile([128, RB], F16, tag="et")
                            nc.scalar.activation(
                                e_t[:], s_bcast[:], AF.Prelu,
                                bias=t_sb[:, jt:jt + 1], scale=1.0, alpha=ALPHA,
                            )
                            v_t = chp.tile([128, RB], F16, tag="vt")
                            nc.vector.tensor_tensor(v_t[:], mt_t, e_t[:], ALU.mult)
                        else:
                            g_t = chp.tile([128, RB], F16, tag="et")
                            nc.vector.scalar_tensor_tensor(
                                g_t[:], s_bcast[:], t_sb[:, jt:jt + 1], mt_t,
                                ALU.add, ALU.mult,
                            )
                            v_t = chp.tile([128, RB], F16, tag="vt")
                            nc.vector.scalar_tensor_tensor(
                                v_t[:], g_t[:], 0.2, g_t[:], ALU.mult, ALU.max,
                            )
                        z_t = chp.tile([128, RB], F16, tag="zt")
                        zeng = nc.gpsimd if z_gp else nc.vector
                        zeng.tensor_scalar(
                            z_t[:], mt_t, 0.0, NEG, ALU.is_equal, ALU.mult
                        )
                        u_t = chp.tile([128, RB], F16, tag="ut")
                        nc.vector.tensor_tensor(u_t[:], v_t[:], z_t[:], ALU.add)
                        p_t = ppool.tile([128, RB], F16, tag="pt")
                        nc.scalar.activation(p_t[:], u_t[:], AF.Exp)

                        # PSUM bank limit: 512 f32 free-dim per matmul;
                        # both halves per lhsT so the stationary operand
                        # loads once per pair
                        for ih in range(2):
                            nc.tensor.matmul(
                                rsT[:, ih * 512:(ih + 1) * 512],
                                onescol[:], p_t[:, ih * 512:(ih + 1) * 512],
                                start=(jt == 0), stop=(jt == NJT - 1),
                            )
                        for fc in range(2):
                            for ih in range(2):
                                nc.tensor.matmul(
                                    zps[fc][:, ih * 512:(ih + 1) * 512],
                                    h_sb[:, jt, fc * 128:(fc + 1) * 128],
                                    p_t[:, ih * 512:(ih + 1) * 512],
                                    start=(jt == 0), stop=(jt == NJT - 1),
                                )

                # ---- finale
                rs_row = fin.tile([1, RB], F32, tag="rsrow")
                nc.vector.tensor_copy(rs_row[:], rsT[:])
                rtot = misc.tile([128, NIT], F32, tag="rtot")
                for it in range(NIT):
                    prs = hpool.tile([128, 2], F32, tag="hps2")
                    nc.tensor.matmul(
                        prs[:, 0:1],
                        rs_row[0:1, it * 128:(it + 1) * 128], one11[:],
                    )
                    nc.vector.tensor_copy(rtot[:, it:it + 1], prs[:, 0:1])
                rec = misc.tile([128, NIT], F32, tag="rec")
                nc.vector.reciprocal(rec[:], rtot[:])
                # sigmoid(x) = 0.5 + 0.5*tanh(x/2); tanh shares the exp
                # table set (no ACT table reload mid-rep)
                rec2 = misc.tile([128, NIT], F32, tag="rec2")
                nc.vector.tensor_scalar(
                    rec2[:], rec[:], 0.5, None, ALU.mult)

                zt_sb = []
                for fc in range(2):
                    zt = fin.tile([128, RB], F32, tag=f"ztsb{fc}")
                    nc.vector.tensor_copy(zt[:], zps[fc][:])
                    zt_sb.append(zt)
                for it in range(NIT):
                    pz = hpool.tile([128, 258], F32, tag="hps2")
                    for fc in range(2):
                        nc.tensor.matmul(
                            pz[:, fc * 128:(fc + 1) * 128],
                            zt_sb[fc][:, it * 128:(it + 1) * 128],
                            ident32[:],
                        )
                    th = fin2.tile([128, FOUT], F32, tag="th")
                    nc.scalar.activation(
                        th[:], pz[:, :FOUT], AF.Tanh,
                        bias=0.0, scale=rec2[:, it:it + 1],
                    )
                    z_out = fin2.tile([128, FOUT], F32, tag="zout")
                    nc.vector.tensor_scalar(
                        z_out[:], th[:], 0.5, 0.5, ALU.mult, ALU.add)
                    nc.sync.dma_start(z_d[it * 128:(it + 1) * 128, :], z_out[:])
    nc.finalize()
    return nc


# ----------------------------------------------------------------- host glue

_CACHE = {}


def _get_kernel():
    if "v5" not in _CACHE:
        _CACHE["v5"] = build_v5(reps=2)
    return _CACHE["v5"]


def stage_inputs(X, M, W_w, attn_w):
    X = np.ascontiguousarray(X, dtype=np.float32)
    M = np.ascontiguousarray(M, dtype=np.float32)
    W_w = np.ascontiguousarray(W_w, dtype=np.float32)
    attn = np.ascontiguousarray(attn_w, dtype=np.float32).reshape(1, 2 * FOUT)

    XT = np.ascontiguousarray(X.T.astype(np.float16))
    WT = np.ascontiguousarray(W_w.T.astype(np.float16))
    WF = W_w.astype(np.float16)
    reps = np.array([[1]], dtype=np.int32)

    in_maps = []
    for c in range(NCORES):
        MT = np.ascontiguousarray(
            M[c * RB:(c + 1) * RB, :].T.astype(np.float16))
        in_maps.append({
            "mt": MT,
            "xt": XT,
            "xtc": np.ascontiguousarray(XT[:, c * RB:(c + 1) * RB]),
            "wt": WT,
            "wf": WF,
            "attn": attn,
            "reps": reps,
        })
    return in_maps


def kernel(X, M, W_w, attn_w):
    nc = _get_kernel()
    in_maps = stage_inputs(X, M, W_w, attn_w)
    r = run_bass_kernel_spmd(nc, in_maps, list(range(NCORES))).results
    Z = np.concatenate([r[c]["zc"] for c in range(NCORES)], axis=0)
    return Z.astype(np.float32)

